# revision 9
# baseline (speedup 1.0000x reference)
"""Trainium2 Bass kernel for nn_MessageFunction (GNN message passing).

Computes msg[b,o,n] = sum_d We[o,d]*e_vw[b,d,n] + sum_d Ww[o,d]*h_w[b,d,n]
                      + (be+bw)[o]
for B=128, D=768, N=256, data-parallel over B across 8 NeuronCores
(16 batches per core).

Design notes (all hardware-measured on trn2):
- fp16 matmuls with fp32 PSUM accumulation: full PE rate (f32r runs at
  1.25 cyc/col, fp16 at 1.0), rel err ~3e-4 at K=1536. Host casts the
  weights and activations to fp16; this also halves input HBM traffic.
- e and h are fused on host into one k-major slab [2*KT, 128, BPC*N]
  (the computation is [We Ww] @ [e; h]) so each block's activations
  arrive in a single 1.57MB DMA with 1KB contiguous runs.
- Outputs are written fp16 in m-major slabs [MT, 128, BPC*N] (1KB
  contiguous runs per partition) and reassembled + cast to f32 on host:
  halves store traffic vs f32.
- Loads ride the sync HWDGE ring, stores the scalar ring: HWDGE rings
  are FIFO per issuing engine, so stores (which depend on late compute)
  must never queue ahead of the next block's load.
- All weight loads are emitted before the timing loop; weights stay
  resident in SBUF (18KB/partition).
- 8 PSUM banks in flight (bufs=8) for the 576-matmul stream.
- For_i(staggered_reset=True): the default loop places an all-engine
  barrier in the per-iteration reset block, which drains the pipeline;
  staggered reset lets DMA prefetch run across the back-edge.
"""
import numpy as np
import concourse.tile as tile
from concourse import bacc, mybir
from concourse.bass_utils import run_bass_kernel_spmd

try:  # persistent XLA cache: repeated fresh-process runs skip the NEFF compile
    import jax
    jax.config.update("jax_compilation_cache_dir", "/tmp/.jax_kernel_cache")
    jax.config.update("jax_persistent_cache_min_compile_time_secs", 0.5)
except Exception:
    pass

B, D, NN = 128, 768, 256
NCORES = 8
BPC = B // NCORES          # 16 batches per core
PAIR = 2                   # batches per 512-wide moving block
NBLK = BPC // PAIR         # 8 column blocks per pass
NCOL = PAIR * NN           # 512 moving columns
KT = 2 * D // 128          # 12 contraction tiles ([e; h] fused)
MT = D // 128              # 6 output row tiles
F32 = mybir.dt.float32
DT = mybir.dt.float16
NPDT = np.float16


def build(repeat: int = 1, loop_repeat: int = 1, stagger: bool = True,
          xbufs: int = 3, batch_store: bool = False, unroll: int = 4,
          hint_all: bool = False, obufs: int = 6, korder: int = 0,
          explicit_ldw: bool = False):
    nc = bacc.Bacc("TRN2", target_bir_lowering=False, debug=False,
                   num_devices=NCORES)
    # activations arrive host-fused as [2*KT', 128, BPC*NN] fp16 k-slabs
    x = nc.dram_tensor("x", [KT, 128, BPC * NN], DT, kind="ExternalInput").ap()
    wT = nc.dram_tensor("wT", [2 * D, D], DT, kind="ExternalInput").ap()
    bias = nc.dram_tensor("bias", [D], F32, kind="ExternalInput").ap()
    out = nc.dram_tensor("out", [MT, 128, BPC * NN], DT,
                         kind="ExternalOutput").ap()

    wT_v = wT.rearrange("(k p) (m q) -> p k m q", p=128, q=128)  # [128,12,6,128]
    bias_v = bias.rearrange("(m p) -> p m", p=128)               # [128,6]

    with tile.TileContext(nc) as tc:
        with (
            tc.tile_pool(name="wpool", bufs=1) as wpool,
            tc.tile_pool(name="xpool", bufs=xbufs) as xpool,
            tc.tile_pool(name="opool", bufs=obufs) as opool,
            tc.tile_pool(name="pspool", bufs=8, space="PSUM") as pspool,
        ):
            w_t = wpool.tile([128, KT, MT, 128], DT)
            bias_t = wpool.tile([128, MT], F32)
            nc.sync.dma_start(bias_t[:], bias_v)
            nc.sync.dma_start(w_t[:], wT_v)

            def _block(c):
                xt = xpool.tile([128, KT, NCOL], DT, tag="xt", name="xt")
                cs = slice(c * NCOL, (c + 1) * NCOL)
                nc.sync.dma_start(xt[:], x[:, :, cs].rearrange("k p n -> p k n"))
                ot = (opool.tile([128, MT, NCOL], DT, name="ot")
                      if batch_store else None)
                for m in range(MT):
                    ps = pspool.tile([128, NCOL], F32, name="ps")
                    for k in range(KT):
                        nc.tensor.matmul(ps[:], w_t[:, k, m, :], xt[:, k, :],
                                         start=(k == 0), stop=(k == KT - 1))
                    res = ot[:, m, :] if batch_store else opool.tile(
                        [128, NCOL], DT, name="res")[:]
                    nc.scalar.activation(
                        res, ps[:], mybir.ActivationFunctionType.Identity,
                        bias=bias_t[:, m:m + 1], scale=1.0)
                    if not batch_store:
                        nc.scalar.dma_start(out[m, :, cs], res)
                if batch_store:
                    nc.scalar.dma_start(
                        out[:, :, cs].rearrange("m p n -> p m n"), ot[:])

            def _khalf(h, nb):
                # k-outer order: nb blocks share each weight tile, so the
                # PE sees nb consecutive matmuls per LDWEIGHTS content.
                xt = xpool.tile([128, KT, nb * NCOL], DT, tag="xt", name="xt")
                cs = slice(h * nb * NCOL, (h + 1) * nb * NCOL)
                nc.sync.dma_start(xt[:], x[:, :, cs].rearrange("k p n -> p k n"))
                for m in range(MT):
                    pss = [pspool.tile([128, NCOL], F32, name="ps")
                           for _ in range(nb)]
                    for k in range(KT):
                        if explicit_ldw:
                            nc.tensor.ldweights(w_t[:, k, m, :])
                        for c in range(nb):
                            nc.tensor.matmul(
                                pss[c][:], w_t[:, k, m, :],
                                xt[:, k, c * NCOL:(c + 1) * NCOL],
                                start=(k == 0), stop=(k == KT - 1))
                    for c in range(nb):
                        res = opool.tile([128, NCOL], DT, name="res")
                        nc.scalar.activation(
                            res[:], pss[c][:],
                            mybir.ActivationFunctionType.Identity,
                            bias=bias_t[:, m:m + 1], scale=1.0)
                        nc.scalar.dma_start(
                            out[m, :, (h * nb + c) * NCOL:
                                (h * nb + c + 1) * NCOL], res[:])

            def body():
                for _ in range(repeat):
                    if korder:
                        for h in range(NBLK // korder):
                            _khalf(h, korder)
                    else:
                        for c in range(NBLK):
                            _block(c)

            hints = (tuple(mybir.ALL_ENGINES) if hint_all
                     else (mybir.EngineType.PE,))
            if loop_repeat > 1:
                if loop_repeat % unroll:
                    unroll = 1
                with tc.For_i(0, loop_repeat // unroll, 1,
                              staggered_reset=stagger, hint_engines=hints):
                    for _ in range(unroll):
                        body()
            else:
                body()
    nc.compile()
    return nc


def _prep_in_maps(h_w, e_vw, We, be, Ww, bw):
    e_vw = np.asarray(e_vw, dtype=np.float32).astype(NPDT)
    h_w = np.asarray(h_w, dtype=np.float32).astype(NPDT)
    # [We Ww] @ [e; h]: stationary operand is W_cat.T = vstack(We.T, Ww.T)
    wT = np.ascontiguousarray(
        np.concatenate([np.asarray(We, dtype=np.float32).T,
                        np.asarray(Ww, dtype=np.float32).T],
                       axis=0)).astype(NPDT)
    bias = (np.asarray(be, dtype=np.float32)
            + np.asarray(bw, dtype=np.float32)).astype(np.float32)

    kt_half = KT // 2

    def slab(xx, c):
        # [BPC, D, NN] -> [KT/2, 128, BPC*NN] : s[k, p, b*NN+n] = xx[b, k*128+p, n]
        s = xx[c * BPC:(c + 1) * BPC].reshape(BPC, kt_half, 128, NN)
        return s.transpose(1, 2, 0, 3).reshape(kt_half, 128, BPC * NN)

    return [
        {"x": np.ascontiguousarray(
            np.concatenate([slab(e_vw, c), slab(h_w, c)], axis=0)),
         "wT": wT, "bias": bias}
        for c in range(NCORES)
    ]


def _unpack_out(o):
    # [MT, 128, NBLK*PAIR*NN] fp16 -> [BPC, D, NN] f32
    # o[m, p, c*NCOL + pb*NN + n] = msg[c*PAIR+pb, m*128+p, n]
    return np.ascontiguousarray(
        o.reshape(MT, 128, NBLK, PAIR, NN)
         .transpose(2, 3, 0, 1, 4)
         .reshape(BPC, D, NN)).astype(np.float32)


_NC_CACHE = []


def kernel(h_v, h_w, e_vw, We, be, Ww, bw):
    if not _NC_CACHE:
        _NC_CACHE.append(build())
    nc = _NC_CACHE[0]
    in_maps = _prep_in_maps(h_w, e_vw, We, be, Ww, bw)
    r = run_bass_kernel_spmd(nc, in_maps, core_ids=list(range(NCORES)))
    return np.concatenate(
        [_unpack_out(r.results[c]["out"]) for c in range(NCORES)], axis=0)


# revision 10
# speedup vs baseline: 1.1309x; 1.1309x over previous
"""Trainium2 Bass kernel for nn_MessageFunction (GNN message passing).

Computes msg[b,o,n] = sum_d We[o,d]*e_vw[b,d,n] + sum_d Ww[o,d]*h_w[b,d,n]
                      + (be+bw)[o]
for B=128, D=768, N=256, data-parallel over B across 8 NeuronCores
(16 batches per core).

Design notes (all hardware-measured on trn2):
- fp16 matmuls with fp32 PSUM accumulation: full PE rate (f32r runs at
  1.25 cyc/col, fp16 at 1.0), rel err ~3e-4 at K=1536. Host casts the
  weights and activations to fp16; this also halves input HBM traffic.
- e and h are fused on host into one k-major slab [2*KT, 128, BPC*N]
  (the computation is [We Ww] @ [e; h]) so each block's activations
  arrive in a single 1.57MB DMA with 1KB contiguous runs.
- Outputs are written fp16 in m-major slabs [MT, 128, BPC*N] (1KB
  contiguous runs per partition) and reassembled + cast to f32 on host:
  halves store traffic vs f32.
- Loads ride the sync HWDGE ring, stores the scalar ring: HWDGE rings
  are FIFO per issuing engine, so stores (which depend on late compute)
  must never queue ahead of the next block's load.
- All weight loads are emitted before the timing loop; weights stay
  resident in SBUF (18KB/partition).
- 8 PSUM banks in flight (bufs=8) for the 576-matmul stream.
- For_i(staggered_reset=True): the default loop places an all-engine
  barrier in the per-iteration reset block, which drains the pipeline;
  staggered reset lets DMA prefetch run across the back-edge. The body
  is additionally unrolled 4x per For_i iteration (measured -7us/pass
  vs unroll=1 at sustained duty).
- Measured sustained floor for the bare 576-MM stream on this part is
  ~150-154us/pass (PE P0-throttles to ~1.9GHz under continuous load);
  the full kernel runs within ~4us of that floor.
"""
import numpy as np
import concourse.tile as tile
from concourse import bacc, mybir
from concourse.bass_utils import run_bass_kernel_spmd

try:  # persistent XLA cache: repeated fresh-process runs skip the NEFF compile
    import jax
    jax.config.update("jax_compilation_cache_dir", "/tmp/.jax_kernel_cache")
    jax.config.update("jax_persistent_cache_min_compile_time_secs", 0.5)
except Exception:
    pass

B, D, NN = 128, 768, 256
NCORES = 8
BPC = B // NCORES          # 16 batches per core
PAIR = 2                   # batches per 512-wide moving block
NBLK = BPC // PAIR         # 8 column blocks per pass
NCOL = PAIR * NN           # 512 moving columns
KT = 2 * D // 128          # 12 contraction tiles ([e; h] fused)
MT = D // 128              # 6 output row tiles
F32 = mybir.dt.float32
DT = mybir.dt.float16
NPDT = np.float16


def build(repeat: int = 1, loop_repeat: int = 1, stagger: bool = True,
          xbufs: int = 3, batch_store: bool = False, unroll: int = 4,
          hint_all: bool = False, obufs: int = 6, korder: int = 0,
          explicit_ldw: bool = False):
    nc = bacc.Bacc("TRN2", target_bir_lowering=False, debug=False,
                   num_devices=NCORES)
    # activations arrive host-fused as [2*KT', 128, BPC*NN] fp16 k-slabs
    x = nc.dram_tensor("x", [KT, 128, BPC * NN], DT, kind="ExternalInput").ap()
    wT = nc.dram_tensor("wT", [2 * D, D], DT, kind="ExternalInput").ap()
    bias = nc.dram_tensor("bias", [D], F32, kind="ExternalInput").ap()
    out = nc.dram_tensor("out", [MT, 128, BPC * NN], DT,
                         kind="ExternalOutput").ap()

    wT_v = wT.rearrange("(k p) (m q) -> p k m q", p=128, q=128)  # [128,12,6,128]
    bias_v = bias.rearrange("(m p) -> p m", p=128)               # [128,6]

    with tile.TileContext(nc) as tc:
        with (
            tc.tile_pool(name="wpool", bufs=1) as wpool,
            tc.tile_pool(name="xpool", bufs=xbufs) as xpool,
            tc.tile_pool(name="opool", bufs=obufs) as opool,
            tc.tile_pool(name="pspool", bufs=8, space="PSUM") as pspool,
        ):
            w_t = wpool.tile([128, KT, MT, 128], DT)
            bias_t = wpool.tile([128, MT], F32)
            nc.sync.dma_start(bias_t[:], bias_v)
            nc.sync.dma_start(w_t[:], wT_v)

            def _block(c):
                xt = xpool.tile([128, KT, NCOL], DT, tag="xt", name="xt")
                cs = slice(c * NCOL, (c + 1) * NCOL)
                nc.sync.dma_start(xt[:], x[:, :, cs].rearrange("k p n -> p k n"))
                ot = (opool.tile([128, MT, NCOL], DT, name="ot")
                      if batch_store else None)
                for m in range(MT):
                    ps = pspool.tile([128, NCOL], F32, name="ps")
                    for k in range(KT):
                        nc.tensor.matmul(ps[:], w_t[:, k, m, :], xt[:, k, :],
                                         start=(k == 0), stop=(k == KT - 1))
                    res = ot[:, m, :] if batch_store else opool.tile(
                        [128, NCOL], DT, name="res")[:]
                    nc.scalar.activation(
                        res, ps[:], mybir.ActivationFunctionType.Identity,
                        bias=bias_t[:, m:m + 1], scale=1.0)
                    if not batch_store:
                        nc.scalar.dma_start(out[m, :, cs], res)
                if batch_store:
                    nc.scalar.dma_start(
                        out[:, :, cs].rearrange("m p n -> p m n"), ot[:])

            def _khalf(h, nb):
                # k-outer order: nb blocks share each weight tile, so the
                # PE sees nb consecutive matmuls per LDWEIGHTS content.
                xt = xpool.tile([128, KT, nb * NCOL], DT, tag="xt", name="xt")
                cs = slice(h * nb * NCOL, (h + 1) * nb * NCOL)
                nc.sync.dma_start(xt[:], x[:, :, cs].rearrange("k p n -> p k n"))
                for m in range(MT):
                    pss = [pspool.tile([128, NCOL], F32, name="ps")
                           for _ in range(nb)]
                    for k in range(KT):
                        if explicit_ldw:
                            nc.tensor.ldweights(w_t[:, k, m, :])
                        for c in range(nb):
                            nc.tensor.matmul(
                                pss[c][:], w_t[:, k, m, :],
                                xt[:, k, c * NCOL:(c + 1) * NCOL],
                                start=(k == 0), stop=(k == KT - 1))
                    for c in range(nb):
                        res = opool.tile([128, NCOL], DT, name="res")
                        nc.scalar.activation(
                            res[:], pss[c][:],
                            mybir.ActivationFunctionType.Identity,
                            bias=bias_t[:, m:m + 1], scale=1.0)
                        nc.scalar.dma_start(
                            out[m, :, (h * nb + c) * NCOL:
                                (h * nb + c + 1) * NCOL], res[:])

            def body():
                for _ in range(repeat):
                    if korder:
                        for h in range(NBLK // korder):
                            _khalf(h, korder)
                    else:
                        for c in range(NBLK):
                            _block(c)

            hints = (tuple(mybir.ALL_ENGINES) if hint_all
                     else (mybir.EngineType.PE,))
            if loop_repeat > 1:
                if loop_repeat % unroll:
                    unroll = 1
                with tc.For_i(0, loop_repeat // unroll, 1,
                              staggered_reset=stagger, hint_engines=hints):
                    for _ in range(unroll):
                        body()
            else:
                body()
    nc.compile()
    return nc


def _prep_in_maps(h_w, e_vw, We, be, Ww, bw):
    e_vw = np.asarray(e_vw, dtype=np.float32).astype(NPDT)
    h_w = np.asarray(h_w, dtype=np.float32).astype(NPDT)
    # [We Ww] @ [e; h]: stationary operand is W_cat.T = vstack(We.T, Ww.T)
    wT = np.ascontiguousarray(
        np.concatenate([np.asarray(We, dtype=np.float32).T,
                        np.asarray(Ww, dtype=np.float32).T],
                       axis=0)).astype(NPDT)
    bias = (np.asarray(be, dtype=np.float32)
            + np.asarray(bw, dtype=np.float32)).astype(np.float32)

    kt_half = KT // 2

    def slab(xx, c):
        # [BPC, D, NN] -> [KT/2, 128, BPC*NN] : s[k, p, b*NN+n] = xx[b, k*128+p, n]
        s = xx[c * BPC:(c + 1) * BPC].reshape(BPC, kt_half, 128, NN)
        return s.transpose(1, 2, 0, 3).reshape(kt_half, 128, BPC * NN)

    return [
        {"x": np.ascontiguousarray(
            np.concatenate([slab(e_vw, c), slab(h_w, c)], axis=0)),
         "wT": wT, "bias": bias}
        for c in range(NCORES)
    ]


def _unpack_out(o):
    # [MT, 128, NBLK*PAIR*NN] fp16 -> [BPC, D, NN] f32
    # o[m, p, c*NCOL + pb*NN + n] = msg[c*PAIR+pb, m*128+p, n]
    return np.ascontiguousarray(
        o.reshape(MT, 128, NBLK, PAIR, NN)
         .transpose(2, 3, 0, 1, 4)
         .reshape(BPC, D, NN)).astype(np.float32)


_NC_CACHE = []


def kernel(h_v, h_w, e_vw, We, be, Ww, bw):
    if not _NC_CACHE:
        _NC_CACHE.append(build())
    nc = _NC_CACHE[0]
    in_maps = _prep_in_maps(h_w, e_vw, We, be, Ww, bw)
    r = run_bass_kernel_spmd(nc, in_maps, core_ids=list(range(NCORES)))
    return np.concatenate(
        [_unpack_out(r.results[c]["out"]) for c in range(NCORES)], axis=0)


# revision 12
# speedup vs baseline: 1.1367x; 1.0051x over previous
"""Trainium2 Bass kernel for nn_MessageFunction (GNN message passing).

Computes msg[b,o,n] = sum_d We[o,d]*e_vw[b,d,n] + sum_d Ww[o,d]*h_w[b,d,n]
                      + (be+bw)[o]
for B=128, D=768, N=256, data-parallel over B across 8 NeuronCores
(16 batches per core).

Design notes (all hardware-measured on trn2):
- fp16 matmuls with fp32 PSUM accumulation: full PE rate (f32r runs at
  1.25 cyc/col, fp16 at 1.0), rel err ~3e-4 at K=1536. Host casts the
  weights and activations to fp16; this also halves input HBM traffic.
- e and h are fused on host into one k-major slab [2*KT, 128, BPC*N]
  (the computation is [We Ww] @ [e; h]) so each block's activations
  arrive in a single 1.57MB DMA with 1KB contiguous runs.
- Outputs are written fp16 in m-major slabs [MT, 128, BPC*N] (1KB
  contiguous runs per partition) and reassembled + cast to f32 on host:
  halves store traffic vs f32.
- Loads ride the sync HWDGE ring, stores the scalar ring: HWDGE rings
  are FIFO per issuing engine, so stores (which depend on late compute)
  must never queue ahead of the next block's load.
- All weight loads are emitted before the timing loop; weights stay
  resident in SBUF (18KB/partition).
- 8 PSUM banks in flight (bufs=8) for the 576-matmul stream.
- For_i(staggered_reset=True): the default loop places an all-engine
  barrier in the per-iteration reset block, which drains the pipeline;
  staggered reset lets DMA prefetch run across the back-edge. The body
  is additionally unrolled 4x per For_i iteration (measured -7us/pass
  vs unroll=1 at sustained duty).
- Measured sustained floor for the bare 576-MM stream on this part is
  ~150-154us/pass (PE P0-throttles to ~1.9GHz under continuous load);
  the full kernel runs within ~4us of that floor.
"""
import numpy as np
import concourse.tile as tile
from concourse import bacc, mybir
from concourse.bass_utils import run_bass_kernel_spmd

try:  # persistent XLA cache: repeated fresh-process runs skip the NEFF compile
    import jax
    jax.config.update("jax_compilation_cache_dir", "/tmp/.jax_kernel_cache")
    jax.config.update("jax_persistent_cache_min_compile_time_secs", 0.5)
except Exception:
    pass

B, D, NN = 128, 768, 256
NCORES = 8
BPC = B // NCORES          # 16 batches per core
PAIR = 2                   # batches per 512-wide moving block
NBLK = BPC // PAIR         # 8 column blocks per pass
NCOL = PAIR * NN           # 512 moving columns
KT = 2 * D // 128          # 12 contraction tiles ([e; h] fused)
MT = D // 128              # 6 output row tiles
F32 = mybir.dt.float32
DT = mybir.dt.float16
NPDT = np.float16


def build(repeat: int = 1, loop_repeat: int = 1, stagger: bool = True,
          xbufs: int = 3, batch_store: bool = False, unroll: int = 4,
          hint_all: bool = False, obufs: int = 6, korder: int = 0,
          explicit_ldw: bool = False, dve_split: bool = False):
    nc = bacc.Bacc("TRN2", target_bir_lowering=False, debug=False,
                   num_devices=NCORES)
    # activations arrive host-fused as [2*KT', 128, BPC*NN] fp16 k-slabs
    x = nc.dram_tensor("x", [KT, 128, BPC * NN], DT, kind="ExternalInput").ap()
    wT = nc.dram_tensor("wT", [2 * D, D], DT, kind="ExternalInput").ap()
    bias = nc.dram_tensor("bias", [D], F32, kind="ExternalInput").ap()
    out = nc.dram_tensor("out", [MT, 128, BPC * NN], DT,
                         kind="ExternalOutput").ap()

    wT_v = wT.rearrange("(k p) (m q) -> p k m q", p=128, q=128)  # [128,12,6,128]
    bias_v = bias.rearrange("(m p) -> p m", p=128)               # [128,6]

    with tile.TileContext(nc) as tc:
        with (
            tc.tile_pool(name="wpool", bufs=1) as wpool,
            tc.tile_pool(name="xpool", bufs=xbufs) as xpool,
            tc.tile_pool(name="opool", bufs=obufs) as opool,
            tc.tile_pool(name="pspool", bufs=8, space="PSUM") as pspool,
        ):
            w_t = wpool.tile([128, KT, MT, 128], DT)
            bias_t = wpool.tile([128, MT], F32)
            nc.sync.dma_start(bias_t[:], bias_v)
            nc.sync.dma_start(w_t[:], wT_v)

            def _block(c):
                xt = xpool.tile([128, KT, NCOL], DT, tag="xt", name="xt")
                cs = slice(c * NCOL, (c + 1) * NCOL)
                nc.sync.dma_start(xt[:], x[:, :, cs].rearrange("k p n -> p k n"))
                ot = (opool.tile([128, MT, NCOL], DT, name="ot")
                      if batch_store else None)
                for m in range(MT):
                    ps = pspool.tile([128, NCOL], F32, name="ps")
                    for k in range(KT):
                        nc.tensor.matmul(ps[:], w_t[:, k, m, :], xt[:, k, :],
                                         start=(k == 0), stop=(k == KT - 1))
                    res = ot[:, m, :] if batch_store else opool.tile(
                        [128, NCOL], DT, name="res")[:]
                    if dve_split and (m % 2 == 1):
                        # odd m: drain on the (otherwise idle) DVE so the
                        # ACT engine isn't the sole PSUM-drain path; store
                        # rides the sync ring to avoid stalling ACT's
                        # HWDGE queue on a cross-engine wait.
                        nc.vector.tensor_scalar_add(
                            res, ps[:], bias_t[:, m:m + 1])
                        if not batch_store:
                            nc.sync.dma_start(out[m, :, cs], res)
                    else:
                        nc.scalar.activation(
                            res, ps[:], mybir.ActivationFunctionType.Identity,
                            bias=bias_t[:, m:m + 1], scale=1.0)
                        if not batch_store:
                            nc.scalar.dma_start(out[m, :, cs], res)
                if batch_store:
                    nc.scalar.dma_start(
                        out[:, :, cs].rearrange("m p n -> p m n"), ot[:])

            def _khalf(h, nb):
                # k-outer order: nb blocks share each weight tile, so the
                # PE sees nb consecutive matmuls per LDWEIGHTS content.
                xt = xpool.tile([128, KT, nb * NCOL], DT, tag="xt", name="xt")
                cs = slice(h * nb * NCOL, (h + 1) * nb * NCOL)
                nc.sync.dma_start(xt[:], x[:, :, cs].rearrange("k p n -> p k n"))
                for m in range(MT):
                    pss = [pspool.tile([128, NCOL], F32, name="ps")
                           for _ in range(nb)]
                    for k in range(KT):
                        if explicit_ldw:
                            nc.tensor.ldweights(w_t[:, k, m, :])
                        for c in range(nb):
                            nc.tensor.matmul(
                                pss[c][:], w_t[:, k, m, :],
                                xt[:, k, c * NCOL:(c + 1) * NCOL],
                                start=(k == 0), stop=(k == KT - 1))
                    for c in range(nb):
                        res = opool.tile([128, NCOL], DT, name="res")
                        nc.scalar.activation(
                            res[:], pss[c][:],
                            mybir.ActivationFunctionType.Identity,
                            bias=bias_t[:, m:m + 1], scale=1.0)
                        nc.scalar.dma_start(
                            out[m, :, (h * nb + c) * NCOL:
                                (h * nb + c + 1) * NCOL], res[:])

            def body():
                for _ in range(repeat):
                    if korder:
                        for h in range(NBLK // korder):
                            _khalf(h, korder)
                    else:
                        for c in range(NBLK):
                            _block(c)

            hints = (tuple(mybir.ALL_ENGINES) if hint_all
                     else (mybir.EngineType.PE,))
            if loop_repeat > 1:
                if loop_repeat % unroll:
                    unroll = 1
                with tc.For_i(0, loop_repeat // unroll, 1,
                              staggered_reset=stagger, hint_engines=hints):
                    for _ in range(unroll):
                        body()
            else:
                body()
    nc.compile()
    return nc


def _prep_in_maps(h_w, e_vw, We, be, Ww, bw):
    e_vw = np.asarray(e_vw, dtype=np.float32).astype(NPDT)
    h_w = np.asarray(h_w, dtype=np.float32).astype(NPDT)
    # [We Ww] @ [e; h]: stationary operand is W_cat.T = vstack(We.T, Ww.T)
    wT = np.ascontiguousarray(
        np.concatenate([np.asarray(We, dtype=np.float32).T,
                        np.asarray(Ww, dtype=np.float32).T],
                       axis=0)).astype(NPDT)
    bias = (np.asarray(be, dtype=np.float32)
            + np.asarray(bw, dtype=np.float32)).astype(np.float32)

    kt_half = KT // 2

    def slab(xx, c):
        # [BPC, D, NN] -> [KT/2, 128, BPC*NN] : s[k, p, b*NN+n] = xx[b, k*128+p, n]
        s = xx[c * BPC:(c + 1) * BPC].reshape(BPC, kt_half, 128, NN)
        return s.transpose(1, 2, 0, 3).reshape(kt_half, 128, BPC * NN)

    return [
        {"x": np.ascontiguousarray(
            np.concatenate([slab(e_vw, c), slab(h_w, c)], axis=0)),
         "wT": wT, "bias": bias}
        for c in range(NCORES)
    ]


def _unpack_out(o):
    # [MT, 128, NBLK*PAIR*NN] fp16 -> [BPC, D, NN] f32
    # o[m, p, c*NCOL + pb*NN + n] = msg[c*PAIR+pb, m*128+p, n]
    return np.ascontiguousarray(
        o.reshape(MT, 128, NBLK, PAIR, NN)
         .transpose(2, 3, 0, 1, 4)
         .reshape(BPC, D, NN)).astype(np.float32)


_NC_CACHE = []


def kernel(h_v, h_w, e_vw, We, be, Ww, bw):
    if not _NC_CACHE:
        _NC_CACHE.append(build())
    nc = _NC_CACHE[0]
    in_maps = _prep_in_maps(h_w, e_vw, We, be, Ww, bw)
    r = run_bass_kernel_spmd(nc, in_maps, core_ids=list(range(NCORES)))
    return np.concatenate(
        [_unpack_out(r.results[c]["out"]) for c in range(NCORES)], axis=0)


# revision 15
# speedup vs baseline: 1.1699x; 1.0292x over previous
"""Trainium2 Bass kernel for nn_MessageFunction (GNN message passing).

Computes msg[b,o,n] = sum_d We[o,d]*e_vw[b,d,n] + sum_d Ww[o,d]*h_w[b,d,n]
                      + (be+bw)[o]
for B=128, D=768, N=256, data-parallel over B across 8 NeuronCores
(16 batches per core).

Design notes (all hardware-measured on trn2):
- fp16 matmuls with fp32 PSUM accumulation: full PE rate (f32r runs at
  1.25 cyc/col, fp16 at 1.0), rel err ~3e-4 at K=1536. Host casts the
  weights and activations to fp16; this also halves input HBM traffic.
- e and h are fused on host into one k-major slab [2*KT, 128, BPC*N]
  (the computation is [We Ww] @ [e; h]) so each block's activations
  arrive in a single 1.57MB DMA with 1KB contiguous runs.
- Outputs are written fp16 in m-major slabs [MT, 128, BPC*N] (1KB
  contiguous runs per partition) and reassembled + cast to f32 on host:
  halves store traffic vs f32.
- Loads ride the sync HWDGE ring, stores the scalar ring: HWDGE rings
  are FIFO per issuing engine, so stores (which depend on late compute)
  must never queue ahead of the next block's load.
- All weight loads are emitted before the timing loop; weights stay
  resident in SBUF (18KB/partition).
- 8 PSUM banks in flight (bufs=8) for the 576-matmul stream.
- For_i(staggered_reset=True): the default loop places an all-engine
  barrier in the per-iteration reset block, which drains the pipeline;
  staggered reset lets DMA prefetch run across the back-edge. The body
  is additionally unrolled 4x per For_i iteration (measured -7us/pass
  vs unroll=1 at sustained duty).
- Measured sustained floor for the bare 576-MM stream on this part is
  ~150-154us/pass (PE P0-throttles to ~1.9GHz under continuous load);
  the full kernel runs within ~4us of that floor.
"""
import numpy as np
import concourse.tile as tile
from concourse import bacc, mybir
from concourse.bass_utils import run_bass_kernel_spmd

try:  # persistent XLA cache: repeated fresh-process runs skip the NEFF compile
    import jax
    jax.config.update("jax_compilation_cache_dir", "/tmp/.jax_kernel_cache")
    jax.config.update("jax_persistent_cache_min_compile_time_secs", 0.5)
except Exception:
    pass

B, D, NN = 128, 768, 256
NCORES = 8
BPC = B // NCORES          # 16 batches per core
PAIR = 2                   # batches per 512-wide moving block
NBLK = BPC // PAIR         # 8 column blocks per pass
NCOL = PAIR * NN           # 512 moving columns
KT = 2 * D // 128          # 12 contraction tiles ([e; h] fused)
MT = D // 128              # 6 output row tiles
F32 = mybir.dt.float32
DT = mybir.dt.float16
NPDT = np.float16


def build(repeat: int = 1, loop_repeat: int = 1, stagger: bool = True,
          xbufs: int = 3, batch_store: bool = False, unroll: int = 4,
          hint_all: bool = False, obufs: int = 6, korder: int = 0,
          explicit_ldw: bool = False, dve_split: bool = False,
          bf16: bool = False):
    adt = mybir.dt.bfloat16 if bf16 else DT
    nc = bacc.Bacc("TRN2", target_bir_lowering=False, debug=False,
                   num_devices=NCORES)
    # activations arrive host-fused as [2*KT', 128, BPC*NN] fp16 k-slabs
    x = nc.dram_tensor("x", [KT, 128, BPC * NN], adt, kind="ExternalInput").ap()
    wT = nc.dram_tensor("wT", [2 * D, D], adt, kind="ExternalInput").ap()
    bias = nc.dram_tensor("bias", [D], F32, kind="ExternalInput").ap()
    out = nc.dram_tensor("out", [MT, 128, BPC * NN], DT,
                         kind="ExternalOutput").ap()

    wT_v = wT.rearrange("(k p) (m q) -> p k m q", p=128, q=128)  # [128,12,6,128]
    bias_v = bias.rearrange("(m p) -> p m", p=128)               # [128,6]

    with tile.TileContext(nc) as tc:
        with (
            tc.tile_pool(name="wpool", bufs=1) as wpool,
            tc.tile_pool(name="xpool", bufs=xbufs) as xpool,
            tc.tile_pool(name="opool", bufs=obufs) as opool,
            tc.tile_pool(name="pspool", bufs=8, space="PSUM") as pspool,
        ):
            w_t = wpool.tile([128, KT, MT, 128], adt)
            bias_t = wpool.tile([128, MT], F32)
            nc.sync.dma_start(bias_t[:], bias_v)
            nc.sync.dma_start(w_t[:], wT_v)

            def _block(c):
                xt = xpool.tile([128, KT, NCOL], adt, tag="xt", name="xt")
                cs = slice(c * NCOL, (c + 1) * NCOL)
                nc.sync.dma_start(xt[:], x[:, :, cs].rearrange("k p n -> p k n"))
                ot = (opool.tile([128, MT, NCOL], DT, name="ot")
                      if batch_store else None)
                for m in range(MT):
                    ps = pspool.tile([128, NCOL], F32, name="ps")
                    for k in range(KT):
                        nc.tensor.matmul(ps[:], w_t[:, k, m, :], xt[:, k, :],
                                         start=(k == 0), stop=(k == KT - 1))
                    res = ot[:, m, :] if batch_store else opool.tile(
                        [128, NCOL], DT, name="res")[:]
                    if dve_split and (m % 2 == 1):
                        # odd m: drain on the (otherwise idle) DVE so the
                        # ACT engine isn't the sole PSUM-drain path; store
                        # rides the sync ring to avoid stalling ACT's
                        # HWDGE queue on a cross-engine wait.
                        nc.vector.tensor_scalar_add(
                            res, ps[:], bias_t[:, m:m + 1])
                        if not batch_store:
                            nc.sync.dma_start(out[m, :, cs], res)
                    else:
                        nc.scalar.activation(
                            res, ps[:], mybir.ActivationFunctionType.Identity,
                            bias=bias_t[:, m:m + 1], scale=1.0)
                        if not batch_store:
                            nc.scalar.dma_start(out[m, :, cs], res)
                if batch_store:
                    nc.scalar.dma_start(
                        out[:, :, cs].rearrange("m p n -> p m n"), ot[:])

            def _khalf(h, nb):
                # k-outer order: nb blocks share each weight tile, so the
                # PE sees nb consecutive matmuls per LDWEIGHTS content.
                xt = xpool.tile([128, KT, nb * NCOL], adt, tag="xt", name="xt")
                cs = slice(h * nb * NCOL, (h + 1) * nb * NCOL)
                nc.sync.dma_start(xt[:], x[:, :, cs].rearrange("k p n -> p k n"))
                for m in range(MT):
                    pss = [pspool.tile([128, NCOL], F32, name="ps")
                           for _ in range(nb)]
                    for k in range(KT):
                        if explicit_ldw:
                            nc.tensor.ldweights(w_t[:, k, m, :])
                        for c in range(nb):
                            nc.tensor.matmul(
                                pss[c][:], w_t[:, k, m, :],
                                xt[:, k, c * NCOL:(c + 1) * NCOL],
                                start=(k == 0), stop=(k == KT - 1))
                    for c in range(nb):
                        res = opool.tile([128, NCOL], DT, name="res")
                        nc.scalar.activation(
                            res[:], pss[c][:],
                            mybir.ActivationFunctionType.Identity,
                            bias=bias_t[:, m:m + 1], scale=1.0)
                        nc.scalar.dma_start(
                            out[m, :, (h * nb + c) * NCOL:
                                (h * nb + c + 1) * NCOL], res[:])

            def body():
                for _ in range(repeat):
                    if korder:
                        for h in range(NBLK // korder):
                            _khalf(h, korder)
                    else:
                        for c in range(NBLK):
                            _block(c)

            hints = (tuple(mybir.ALL_ENGINES) if hint_all
                     else (mybir.EngineType.PE,))
            if loop_repeat > 1:
                if loop_repeat % unroll:
                    unroll = 1
                with tc.For_i(0, loop_repeat // unroll, 1,
                              staggered_reset=stagger, hint_engines=hints):
                    for _ in range(unroll):
                        body()
            else:
                body()
    nc.compile()
    return nc


def _prep_in_maps(h_w, e_vw, We, be, Ww, bw, bf16=False):
    if bf16:
        import ml_dtypes
        npdt = np.dtype(ml_dtypes.bfloat16)
    else:
        npdt = NPDT
    e_vw = np.asarray(e_vw, dtype=np.float32).astype(npdt)
    h_w = np.asarray(h_w, dtype=np.float32).astype(npdt)
    # [We Ww] @ [e; h]: stationary operand is W_cat.T = vstack(We.T, Ww.T)
    wT = np.ascontiguousarray(
        np.concatenate([np.asarray(We, dtype=np.float32).T,
                        np.asarray(Ww, dtype=np.float32).T],
                       axis=0)).astype(npdt)
    bias = (np.asarray(be, dtype=np.float32)
            + np.asarray(bw, dtype=np.float32)).astype(np.float32)

    kt_half = KT // 2

    def slab(xx, c):
        # [BPC, D, NN] -> [KT/2, 128, BPC*NN] : s[k, p, b*NN+n] = xx[b, k*128+p, n]
        s = xx[c * BPC:(c + 1) * BPC].reshape(BPC, kt_half, 128, NN)
        return s.transpose(1, 2, 0, 3).reshape(kt_half, 128, BPC * NN)

    return [
        {"x": np.ascontiguousarray(
            np.concatenate([slab(e_vw, c), slab(h_w, c)], axis=0)),
         "wT": wT, "bias": bias}
        for c in range(NCORES)
    ]


def _unpack_out(o):
    # [MT, 128, NBLK*PAIR*NN] fp16 -> [BPC, D, NN] f32
    # o[m, p, c*NCOL + pb*NN + n] = msg[c*PAIR+pb, m*128+p, n]
    return np.ascontiguousarray(
        o.reshape(MT, 128, NBLK, PAIR, NN)
         .transpose(2, 3, 0, 1, 4)
         .reshape(BPC, D, NN)).astype(np.float32)


_NC_CACHE = []


def kernel(h_v, h_w, e_vw, We, be, Ww, bw):
    if not _NC_CACHE:
        _NC_CACHE.append(build())
    nc = _NC_CACHE[0]
    in_maps = _prep_in_maps(h_w, e_vw, We, be, Ww, bw)
    r = run_bass_kernel_spmd(nc, in_maps, core_ids=list(range(NCORES)))
    return np.concatenate(
        [_unpack_out(r.results[c]["out"]) for c in range(NCORES)], axis=0)
